# revision 2
# baseline (speedup 1.0000x reference)
"""DGCNN layer (dynamic kNN graph + edge MLP) for 8 Trainium2 cores.

Algorithm per core (node-sharded, 2048 target rows each):
  1. Score matmul on PE (fp32, exact): v[i,j] = 2*x_i.x_j - |x_j|^2
     (rank-equivalent to -dist; row-constant |x_i|^2 dropped) into
     [128, 1024] PSUM windows, two 512-wide matmuls per window.
  2. Top-16 screen on DVE straight from PSUM: per 1024-wide window, Max8
     top-8 values + their in-window indices. 16 windows x 8 = 128 coarse
     candidates per row (a window holding >8 of a row's true top-16 has
     probability ~1e-6 per row; none occur for this input).
  3. Merge without per-partition gathers: two max8+match_replace rounds mark
     the top-16 coarse slots in-place, then a re-max over mask*2^20 +
     globalidx compacts the winning indices.
  4. Edge MLP: neighbor features x_j are fetched by SWDGE dma_gather (512
     indices per call — larger calls overflow the 1024-slot descriptor ring)
     from an HBM fp16 table [N, 128] (x_j zero-padded to 256B rows), landing
     transposed as matmul-ready [features, edges] tiles. Layer 1 is
     relu(W1b @ x_j + p_i + b1): the p_i term rides a second accumulating
     matmul with a 0/1 row-selector, p = x @ (W1a - W1b) precomputed on PE.
     Layer 2 is an fp16 matmul; relu/bias/mean-scale fused into the ACT
     evacuation; the mean over 16 neighbors is an fp16 tensor-tensor add
     tree on DVE (2x mode).
Block-level software pipeline with offset 2: block b's screen overlaps block
b-1's index transpose (PE transpose via fp32 identity) + gathers and block
b-2's MLP, so the DVE screen scan is the sole critical path (~87% DVE busy).
Output is produced transposed [C, rows]; the host transposes back.
"""

import os
import sys

import numpy as np

N, D, C, K = 16384, 64, 128, 16
NCORES = 8
RPC = N // NCORES          # rows per core
BLK = 128                  # target rows per screen block
WIN = 1024                 # screen window (2 PSUM banks of fp32)
MMW = 512                  # matmul sub-window / MLP chunk / gather width
DA = D + 2                 # augmented contraction dim

_REPO = "/opt/trn_rl_repo"


def _ensure_path():
    if _REPO not in sys.path:
        sys.path.insert(0, _REPO)


def build_program(n=N, d=D, c=C, k=K, rpc=RPC):
    _ensure_path()
    import concourse.mybir as mybir
    from concourse import tile
    from concourse.bacc import Bacc

    f32 = mybir.dt.float32
    f16 = mybir.dt.float16
    i16 = mybir.dt.int16
    u16 = mybir.dt.uint16

    da = d + 2
    nblk = rpc // BLK                    # 16
    nwin = n // WIN                      # 16
    nco = nwin * 8                       # 128 coarse slots per row
    rows_per_chunk = MMW // k            # 32
    subs = BLK // rows_per_chunk         # 4 MLP sub-chunks per block
    nxch = 4                             # x load chunks

    nc = Bacc()

    xf_d = nc.declare_dram_parameter("xaug", [da, n], f32, isOutput=False)
    wf_d = nc.declare_dram_parameter("wloc", [da, rpc], f32, isOutput=False)
    w1dh_d = nc.declare_dram_parameter("w1dh", [d, c], f32, isOutput=False)
    w1b16_d = nc.declare_dram_parameter("w1b16", [c, c], f16, isOutput=False)
    w2_d = nc.declare_dram_parameter("w2", [c, c], f16, isOutput=False)
    ssel_d = nc.declare_dram_parameter("ssel", [rows_per_chunk, MMW], f16, isOutput=False)
    b1_d = nc.declare_dram_parameter("b1c", [c, 1], f32, isOutput=False)
    b2s_d = nc.declare_dram_parameter("b2s", [c, 1], f32, isOutput=False)
    wbase_d = nc.declare_dram_parameter("wbase", [128, nco], f32, isOutput=False)
    ident_d = nc.declare_dram_parameter("identf", [128, 128], f32, isOutput=False)
    xpad_d = nc.declare_dram_parameter("xpad16", [n, c], f16, isOutput=False)
    out_d = nc.declare_dram_parameter("outT", [c, rpc], f32, isOutput=True)

    NEG = -3.0e38
    MARK = float(1 << 20)

    with tile.TileContext(nc) as tc:
        with (
            tc.tile_pool(name="const", bufs=1) as cpool,
            tc.tile_pool(name="screen", bufs=2) as spool,
            tc.tile_pool(name="small", bufs=2) as mpool,
            tc.tile_pool(name="mlp", bufs=3) as dpool,
            tc.tile_pool(name="psA", bufs=2, space="PSUM") as ppA,
            tc.tile_pool(name="psM", bufs=2, space="PSUM") as ppM,
            tc.tile_pool(name="psJ", bufs=2, space="PSUM") as ppJ,
        ):
            # ---- persistent tiles ----
            xaug = cpool.tile([da, n], f32, tag="xaug")
            wloc = cpool.tile([da, rpc], f32, tag="wloc")
            w1dh = cpool.tile([d, c], f32, tag="w1dh")
            w1b16 = cpool.tile([c, c], f16, tag="w1b16")
            w2 = cpool.tile([c, c], f16, tag="w2")
            ssel = cpool.tile([rows_per_chunk, MMW], f16, tag="ssel")
            b1 = cpool.tile([c, 1], f32, tag="b1")
            b2s = cpool.tile([c, 1], f32, tag="b2s")
            wbase = cpool.tile([128, nco], f32, tag="wbase")
            ident = cpool.tile([128, 128], f32, tag="identf")
            prow16 = cpool.tile([rows_per_chunk, nblk * subs * c], f16, tag="prow16")
            outT = cpool.tile([c, rpc], f32, tag="outT")
            jrep = [
                cpool.tile([128, BLK], i16, tag=f"jrep{b}", name=f"jrep{b}")
                for b in range(nblk)
            ]

            nc.sync.dma_start(wloc[:, :], wf_d[:, :])
            nc.sync.dma_start(w1dh[:, :], w1dh_d[:, :])
            # small first slice so window 0's matmuls start early
            xw = n // nxch
            bounds = [0, WIN, xw] + [t * xw for t in range(2, nxch + 1)]
            for lo, hi in zip(bounds[:-1], bounds[1:]):
                nc.sync.dma_start(xaug[:, lo:hi], xf_d[:, lo:hi])
            nc.sync.dma_start(w1b16[:, :], w1b16_d[:, :])
            nc.sync.dma_start(w2[:, :], w2_d[:, :])
            nc.sync.dma_start(ssel[:, :], ssel_d[:, :])
            nc.sync.dma_start(b1[:, :], b1_d[:, :])
            nc.sync.dma_start(b2s[:, :], b2s_d[:, :])
            nc.sync.dma_start(wbase[:, :], wbase_d[:, :])
            nc.sync.dma_start(ident[:, :], ident_d[:, :])

            # ---- p rows for one block's 4 chunks, fp16 out.
            # wloc rows 0:d hold 2*x_loc^T -> stationary; w1dh = 0.5*(W1a-W1b)
            # so p = x_loc @ (W1a - W1b). One matmul per 32-row chunk so rows
            # land at partition base 0 (the mm1 selector accumulate needs
            # lhsT/rhs partition bases to match).
            def phase_p(b):
                for s in range(subs):
                    ch = b * subs + s
                    pp = ppM.tile([128, MMW], f32, tag="mm")
                    r0 = ch * rows_per_chunk
                    nc.tensor.matmul(
                        pp[0:rows_per_chunk, 0:c],
                        wloc[0:d, r0:r0 + rows_per_chunk],
                        w1dh[:, :],
                    )
                    nc.scalar.activation(
                        prow16[:, ch * c:(ch + 1) * c],
                        pp[0:rows_per_chunk, 0:c],
                        mybir.ActivationFunctionType.Copy,
                    )

            jall_tiles = {}
            sc_tiles = {}

            def screen_windows(b, w_lo, w_hi):
                if b not in sc_tiles:
                    sc_tiles[b] = (
                        spool.tile([128, nco], f32, tag="cvals", name=f"cvals{b}"),
                        spool.tile([128, nco], u16, tag="cidx", name=f"cidx{b}"),
                    )
                cvals, cidx = sc_tiles[b]
                rblk = slice(b * BLK, (b + 1) * BLK)
                for w in range(w_lo, w_hi):
                    ps = ppA.tile([128, WIN], f32, tag="scr")
                    for h in range(WIN // MMW):
                        cs = slice(w * WIN + h * MMW, w * WIN + (h + 1) * MMW)
                        nc.tensor.matmul(
                            ps[:, h * MMW:(h + 1) * MMW], wloc[:, rblk], xaug[:, cs]
                        )
                    nc.vector.max(cvals[:, 8 * w:8 * w + 8], ps[:, :])
                    nc.vector.max_index(
                        cidx[:, 8 * w:8 * w + 8], cvals[:, 8 * w:8 * w + 8], ps[:, :]
                    )

            def merge(b):
                cvals, cidx = sc_tiles.pop(b)

                # global candidate index per coarse slot (copy on ACT)
                gj = mpool.tile([128, nco], f32, tag="gj")
                nc.scalar.activation(
                    gj[:, :], cidx[:, :], mybir.ActivationFunctionType.Copy
                )
                nc.vector.tensor_add(gj[:, :], gj[:, :], wbase[:, :])

                # mark top-16 coarse slots in-place
                m8a = mpool.tile([128, 8], f32, tag="m8a")
                m8b = mpool.tile([128, 8], f32, tag="m8b")
                zap = mpool.tile([128, nco], f32, tag="zap")
                nc.vector.max(m8a[:, :], cvals[:, :])
                nc.vector.match_replace(zap[:, :], m8a[:, :], cvals[:, :], NEG)
                nc.vector.max(m8b[:, :], zap[:, :])
                nc.vector.match_replace(zap[:, :], m8b[:, :], zap[:, :], NEG)

                # compact: packed = 2^20 * is_marked + gj, top-16 of packed
                mask = mpool.tile([128, nco], f32, tag="mask")
                nc.vector.tensor_scalar(
                    mask[:, :], zap[:, :], -1.0e38, MARK,
                    op0=mybir.AluOpType.is_le, op1=mybir.AluOpType.mult,
                )
                nc.vector.tensor_add(mask[:, :], mask[:, :], gj[:, :])
                p8a = mpool.tile([128, 8], f32, tag="p8a")
                p8b = mpool.tile([128, 8], f32, tag="p8b")
                nc.vector.max(p8a[:, :], mask[:, :])
                nc.vector.match_replace(mask[:, :], p8a[:, :], mask[:, :], NEG)
                nc.vector.max(p8b[:, :], mask[:, :])

                jall_b = mpool.tile([128, k], f32, tag="jall")
                nc.scalar.activation(
                    jall_b[:, 0:8], p8a[:, :],
                    mybir.ActivationFunctionType.Copy, bias=-MARK,
                )
                nc.scalar.activation(
                    jall_b[:, 8:16], p8b[:, :],
                    mybir.ActivationFunctionType.Copy, bias=-MARK,
                )
                jall_tiles[b] = jall_b

            # PE-transpose block b's [128 rows, 16] f32 index tile to
            # [16, 128], convert to i16 on ACT, replicate into all 8 Q7 idx
            # groups (HW reads every group), then issue the 4 gathers.
            def jrep_and_gather(b):
                jt = ppJ.tile([k, BLK], f32, tag="jt")
                nc.tensor.transpose(jt[:, :], jall_tiles.pop(b)[:, :], ident[:, :])
                jts = mpool.tile([k, BLK], i16, tag="jts")
                nc.scalar.activation(
                    jts[:, :], jt[:, :], mybir.ActivationFunctionType.Copy
                )
                # last block's replication is on the critical tail: spread it
                # across idle queues; otherwise keep everything on SP so the
                # ACT/Pool queues stay clear for merge/gather work
                qeng = (
                    [nc.sync, nc.scalar, nc.gpsimd]
                    if b == nblk - 1 else [nc.sync]
                )
                for q in range(8):
                    qeng[q % len(qeng)].dma_start(
                        jrep[b][16 * q:16 * q + k, :], jts[:, :]
                    )
                xj = dpool.tile([128, BLK * k], f16, tag="xj")
                for s in range(subs):
                    nc.gpsimd.dma_gather(
                        xj[:, s * MMW:(s + 1) * MMW].rearrange(
                            "p (o e) -> p o e", o=1
                        ),
                        xpad_d[:, :],
                        jrep[b][:, s * rows_per_chunk:(s + 1) * rows_per_chunk],
                        MMW,
                        MMW,
                        c,
                        transpose=True,
                    )
                return xj

            def mlp_mms(b, xj):
                h2 = dpool.tile([128, BLK * k], f16, tag="h2")
                for s in range(subs):
                    e0 = s * MMW
                    ps = ppM.tile([128, MMW], f32, tag="mm")
                    nc.tensor.matmul(
                        ps[:, :], w1b16[:, :], xj[:, e0:e0 + MMW],
                        start=True, stop=False,
                    )
                    nc.tensor.matmul(
                        ps[:, :],
                        prow16[:, (b * subs + s) * c:(b * subs + s + 1) * c],
                        ssel[:, :],
                        start=False, stop=True,
                    )
                    h1 = dpool.tile([128, MMW], f16, tag="h1")
                    nc.scalar.activation(
                        h1[:, :], ps[:, :], mybir.ActivationFunctionType.Relu,
                        bias=b1[:, :],
                    )
                    ps2 = ppM.tile([128, MMW], f32, tag="mm")
                    nc.tensor.matmul(ps2[:, :], w2[:, :], h1[:, :])
                    nc.scalar.activation(
                        h2[:, e0:e0 + MMW], ps2[:, :],
                        mybir.ActivationFunctionType.Relu,
                        bias=b2s[:, :], scale=1.0 / k,
                    )
                return h2

            def mlp_tree(b, h2):
                # mean over k=16 via fp16 TT-add tree (2x DVE mode)
                h2v = h2[:, :].rearrange("p (r k) -> p r k", k=k)
                t1 = dpool.tile([128, BLK * 8], f16, tag="t1")
                t1v = t1[:, :].rearrange("p (r k) -> p r k", k=8)
                nc.vector.tensor_tensor(
                    out=t1v, in0=h2v[:, :, 0:8], in1=h2v[:, :, 8:16],
                    op=mybir.AluOpType.add,
                )
                t2 = dpool.tile([128, BLK * 4], f16, tag="t2")
                t2v = t2[:, :].rearrange("p (r k) -> p r k", k=4)
                nc.vector.tensor_tensor(
                    out=t2v, in0=t1v[:, :, 0:4], in1=t1v[:, :, 4:8],
                    op=mybir.AluOpType.add,
                )
                t3 = dpool.tile([128, BLK * 2], f16, tag="t3")
                t3v = t3[:, :].rearrange("p (r k) -> p r k", k=2)
                nc.vector.tensor_tensor(
                    out=t3v, in0=t2v[:, :, 0:2], in1=t2v[:, :, 2:4],
                    op=mybir.AluOpType.add,
                )
                o_v = outT[:, b * BLK:(b + 1) * BLK].rearrange(
                    "p (r k) -> p r k", k=1
                )
                nc.vector.tensor_tensor(
                    out=o_v, in0=t3v[:, :, 0:1], in1=t3v[:, :, 1:2],
                    op=mybir.AluOpType.add,
                )

            def out_dma(b):
                nc.sync.dma_start(
                    out_d[:, b * BLK:(b + 1) * BLK], outT[:, b * BLK:(b + 1) * BLK]
                )

            # Pipeline (offset 2): iter b runs the MLP matmuls of block b-2
            # (whose gather landed during iter b-1) first — they never wait —
            # then block b's screen; block b-1's transpose + gathers go at
            # the iteration tail (transpose input came from merge(b-1) last
            # iteration, so the PE FIFO again never stalls on them).
            xj_pending = {}
            for b in range(nblk):
                h2_prev = None
                if b >= 2:
                    h2_prev = mlp_mms(b - 2, xj_pending.pop(b - 2))
                screen_windows(b, 0, 2)
                phase_p(b)
                if h2_prev is not None:
                    mlp_tree(b - 2, h2_prev)
                    out_dma(b - 2)
                screen_windows(b, 2, nwin)
                merge(b)
                if b >= 1:
                    xj_pending[b - 1] = jrep_and_gather(b - 1)
            b2 = nblk - 2
            mlp_tree(b2, mlp_mms(b2, xj_pending.pop(b2)))
            out_dma(b2)
            blast = nblk - 1
            mlp_tree(blast, mlp_mms(blast, jrep_and_gather(blast)))
            out_dma(blast)

    nc.finalize()
    return nc


def host_prep(x, W1, b1, W2, b2, n=N, d=D, c=C, k=K, rpc=RPC, ncores=NCORES):
    x = np.ascontiguousarray(np.asarray(x, dtype=np.float32))
    W1 = np.asarray(W1, dtype=np.float32)
    b1 = np.asarray(b1, dtype=np.float32)
    W2 = np.asarray(W2, dtype=np.float32)
    b2 = np.asarray(b2, dtype=np.float32)

    sq = np.sum(x * x, axis=1, dtype=np.float32)
    da = d + 2
    nwin = n // WIN
    nco = nwin * 8
    rows_per_chunk = MMW // k

    xaug = np.zeros((da, n), dtype=np.float32)
    xaug[:d] = x.T
    xaug[d] = sq

    w1dh = ((W1[:d] - W1[d:]) * 0.5).astype(np.float32)
    w1b16 = np.zeros((c, c), dtype=np.float16)
    w1b16[:d] = W1[d:].astype(np.float16)
    w2 = W2.astype(np.float16)
    ssel = np.zeros((rows_per_chunk, MMW), dtype=np.float16)
    for r in range(rows_per_chunk):
        ssel[r, r * k:(r + 1) * k] = 1.0
    b1c = b1.reshape(c, 1).astype(np.float32)
    b2s = (b2 / k).reshape(c, 1).astype(np.float32)
    wbase = np.repeat(
        (np.arange(nwin, dtype=np.float32) * WIN), 8
    )[None, :].repeat(128, axis=0).astype(np.float32)
    wbase = np.ascontiguousarray(wbase[:, :nco])
    identf = np.eye(128, dtype=np.float32)
    xpad16 = np.zeros((n, c), dtype=np.float16)
    xpad16[:, :d] = x.astype(np.float16)

    in_maps = []
    for cid in range(ncores):
        rows = x[cid * rpc:(cid + 1) * rpc]
        wloc = np.empty((da, rpc), dtype=np.float32)
        wloc[:d] = 2.0 * rows.T
        wloc[d:] = -1.0
        in_maps.append(
            dict(
                xaug=xaug, wloc=np.ascontiguousarray(wloc), w1dh=w1dh,
                w1b16=w1b16, w2=w2, ssel=ssel, b1c=b1c, b2s=b2s, wbase=wbase,
                identf=identf, xpad16=xpad16,
            )
        )
    return in_maps


_NC_CACHE = {}


def kernel(x, W1, b1, W2, b2):
    _ensure_path()
    from concourse.bass_utils import run_bass_kernel_spmd

    key = "full"
    if key not in _NC_CACHE:
        _NC_CACHE[key] = build_program()
    nc = _NC_CACHE[key]

    in_maps = host_prep(x, W1, b1, W2, b2)
    res = run_bass_kernel_spmd(
        nc, in_maps, core_ids=list(range(NCORES)),
        trace=bool(int(os.environ.get("DGCNN_TRACE", "0"))),
    )
    out = np.empty((N, C), dtype=np.float32)
    for cid in range(NCORES):
        out[cid * RPC:(cid + 1) * RPC] = res.results[cid]["outT"].T
    if getattr(res, "exec_time_ns", None):
        kernel.last_exec_time_ns = res.exec_time_ns
    return out


kernel.last_exec_time_ns = None


# revision 4
# speedup vs baseline: 1.0090x; 1.0090x over previous
"""DGCNN layer (dynamic kNN graph + edge MLP) for 8 Trainium2 cores.

Algorithm per core (node-sharded, 2048 target rows each):
  1. Score matmul on PE (fp32, exact): v[i,j] = 2*x_i.x_j - |x_j|^2
     (rank-equivalent to -dist; row-constant |x_i|^2 dropped) into
     [128, 1024] PSUM windows, two 512-wide matmuls per window.
  2. Top-16 screen on DVE straight from PSUM: per 1024-wide window, Max8
     top-8 values + their in-window indices. 16 windows x 8 = 128 coarse
     candidates per row (a window holding >8 of a row's true top-16 has
     probability ~1e-6 per row; none occur for this input).
  3. Merge without per-partition gathers: two max8+match_replace rounds mark
     the top-16 coarse slots in-place, then a re-max over mask*2^20 +
     globalidx compacts the winning indices.
  4. Edge MLP: neighbor features x_j are fetched by SWDGE dma_gather (512
     indices per call — larger calls overflow the 1024-slot descriptor ring)
     from an HBM fp16 table [N, 128] (x_j zero-padded to 256B rows), landing
     transposed as matmul-ready [features, edges] tiles. Layer 1 is
     relu(W1b @ x_j + p_i + b1): the p_i term rides a second accumulating
     matmul with a 0/1 row-selector, p = x @ (W1a - W1b) precomputed on PE.
     Layer 2 is an fp16 matmul; relu/bias/mean-scale fused into the ACT
     evacuation; the mean over 16 neighbors is an fp16 tensor-tensor add
     tree on DVE (2x mode).
Block-level software pipeline with offset 2: block b's screen overlaps block
b-1's index transpose (PE transpose via fp32 identity) + gathers and block
b-2's MLP, so the DVE screen scan is the sole critical path (~87% DVE busy).
Output is produced transposed [C, rows]; the host transposes back.
"""

import os
import sys

import numpy as np

N, D, C, K = 16384, 64, 128, 16
NCORES = 8
RPC = N // NCORES          # rows per core
BLK = 128                  # target rows per screen block
WIN = 1024                 # screen window (2 PSUM banks of fp32)
MMW = 512                  # matmul sub-window / MLP chunk / gather width
DA = D + 2                 # augmented contraction dim

_REPO = "/opt/trn_rl_repo"


def _ensure_path():
    if _REPO not in sys.path:
        sys.path.insert(0, _REPO)


def build_program(n=N, d=D, c=C, k=K, rpc=RPC):
    _ensure_path()
    import concourse.mybir as mybir
    from concourse import tile
    from concourse.bacc import Bacc

    f32 = mybir.dt.float32
    f16 = mybir.dt.float16
    i16 = mybir.dt.int16
    u16 = mybir.dt.uint16

    da = d + 2
    nblk = rpc // BLK                    # 16
    nwin = n // WIN                      # 16
    nco = nwin * 8                       # 128 coarse slots per row
    rows_per_chunk = MMW // k            # 32
    subs = BLK // rows_per_chunk         # 4 MLP sub-chunks per block
    nxch = 4                             # x load chunks

    nc = Bacc()

    xf_d = nc.declare_dram_parameter("xaug", [da, n], f32, isOutput=False)
    wf_d = nc.declare_dram_parameter("wloc", [da, rpc], f32, isOutput=False)
    w1dh_d = nc.declare_dram_parameter("w1dh", [d, c], f32, isOutput=False)
    w1b16_d = nc.declare_dram_parameter("w1b16", [c, c], f16, isOutput=False)
    w2_d = nc.declare_dram_parameter("w2", [c, c], f16, isOutput=False)
    ssel_d = nc.declare_dram_parameter("ssel", [rows_per_chunk, MMW], f16, isOutput=False)
    b1_d = nc.declare_dram_parameter("b1c", [c, 1], f32, isOutput=False)
    b2s_d = nc.declare_dram_parameter("b2s", [c, 1], f32, isOutput=False)
    ident_d = nc.declare_dram_parameter("identf", [128, 128], f32, isOutput=False)
    xpad_d = nc.declare_dram_parameter("xpad16", [n, c], f16, isOutput=False)
    out_d = nc.declare_dram_parameter("outT", [c, rpc], f32, isOutput=True)

    NEG = -3.0e38
    MARK = float(1 << 20)

    with tile.TileContext(nc) as tc:
        with (
            tc.tile_pool(name="const", bufs=1) as cpool,
            tc.tile_pool(name="screen", bufs=2) as spool,
            tc.tile_pool(name="small", bufs=2) as mpool,
            tc.tile_pool(name="mlp", bufs=3) as dpool,
            tc.tile_pool(name="psA", bufs=2, space="PSUM") as ppA,
            tc.tile_pool(name="psM", bufs=2, space="PSUM") as ppM,
            tc.tile_pool(name="psJ", bufs=2, space="PSUM") as ppJ,
        ):
            # ---- persistent tiles ----
            xaug = cpool.tile([da, n], f32, tag="xaug")
            wloc = cpool.tile([da, rpc], f32, tag="wloc")
            w1dh = cpool.tile([d, c], f32, tag="w1dh")
            w1b16 = cpool.tile([c, c], f16, tag="w1b16")
            w2 = cpool.tile([c, c], f16, tag="w2")
            ssel = cpool.tile([rows_per_chunk, MMW], f16, tag="ssel")
            b1 = cpool.tile([c, 1], f32, tag="b1")
            b2s = cpool.tile([c, 1], f32, tag="b2s")
            ident = cpool.tile([128, 128], f32, tag="identf")
            prow16 = cpool.tile([rows_per_chunk, nblk * subs * c], f16, tag="prow16")
            outT = cpool.tile([c, rpc], f32, tag="outT")
            jrep = [
                cpool.tile([128, BLK], i16, tag=f"jrep{b}", name=f"jrep{b}")
                for b in range(nblk)
            ]

            nc.sync.dma_start(wloc[:, :], wf_d[:, :])
            nc.sync.dma_start(w1dh[:, :], w1dh_d[:, :])
            # small first slice so window 0's matmuls start early
            xw = n // nxch
            bounds = [0, WIN, xw] + [t * xw for t in range(2, nxch + 1)]
            for lo, hi in zip(bounds[:-1], bounds[1:]):
                nc.sync.dma_start(xaug[:, lo:hi], xf_d[:, lo:hi])
            nc.sync.dma_start(w1b16[:, :], w1b16_d[:, :])
            nc.sync.dma_start(w2[:, :], w2_d[:, :])
            nc.sync.dma_start(ssel[:, :], ssel_d[:, :])
            nc.sync.dma_start(b1[:, :], b1_d[:, :])
            nc.sync.dma_start(b2s[:, :], b2s_d[:, :])
            nc.sync.dma_start(ident[:, :], ident_d[:, :])

            # ---- p rows for one block's 4 chunks, fp16 out.
            # wloc rows 0:d hold 2*x_loc^T -> stationary; w1dh = 0.5*(W1a-W1b)
            # so p = x_loc @ (W1a - W1b). One matmul per 32-row chunk so rows
            # land at partition base 0 (the mm1 selector accumulate needs
            # lhsT/rhs partition bases to match).
            def phase_p(b):
                for s in range(subs):
                    ch = b * subs + s
                    pp = ppM.tile([128, MMW], f32, tag="mm")
                    r0 = ch * rows_per_chunk
                    nc.tensor.matmul(
                        pp[0:rows_per_chunk, 0:c],
                        wloc[0:d, r0:r0 + rows_per_chunk],
                        w1dh[:, :],
                    )
                    nc.scalar.activation(
                        prow16[:, ch * c:(ch + 1) * c],
                        pp[0:rows_per_chunk, 0:c],
                        mybir.ActivationFunctionType.Copy,
                    )

            jall_tiles = {}
            sc_tiles = {}

            def screen_windows(b, w_lo, w_hi):
                if b not in sc_tiles:
                    sc_tiles[b] = (
                        spool.tile([128, nco], f32, tag="cvals", name=f"cvals{b}"),
                        spool.tile([128, nco], u16, tag="cidx", name=f"cidx{b}"),
                        spool.tile([128, nco], f32, tag="gj", name=f"gj{b}"),
                    )
                cvals, cidx, gj = sc_tiles[b]
                rblk = slice(b * BLK, (b + 1) * BLK)
                for w in range(w_lo, w_hi):
                    ps = ppA.tile([128, WIN], f32, tag="scr")
                    for h in range(WIN // MMW):
                        cs = slice(w * WIN + h * MMW, w * WIN + (h + 1) * MMW)
                        nc.tensor.matmul(
                            ps[:, h * MMW:(h + 1) * MMW], wloc[:, rblk], xaug[:, cs]
                        )
                    nc.vector.max(cvals[:, 8 * w:8 * w + 8], ps[:, :])
                    nc.vector.max_index(
                        cidx[:, 8 * w:8 * w + 8], cvals[:, 8 * w:8 * w + 8], ps[:, :]
                    )
                    # global index = in-window index + window base, on ACT
                    nc.scalar.activation(
                        gj[:, 8 * w:8 * w + 8], cidx[:, 8 * w:8 * w + 8],
                        mybir.ActivationFunctionType.Copy, bias=float(w * WIN),
                    )

            def merge(b):
                cvals, cidx, gj = sc_tiles.pop(b)

                # mark top-16 coarse slots in-place
                m8a = mpool.tile([128, 8], f32, tag="m8a")
                m8b = mpool.tile([128, 8], f32, tag="m8b")
                zap = mpool.tile([128, nco], f32, tag="zap")
                nc.vector.max(m8a[:, :], cvals[:, :])
                nc.vector.match_replace(zap[:, :], m8a[:, :], cvals[:, :], NEG)
                nc.vector.max(m8b[:, :], zap[:, :])
                nc.vector.match_replace(zap[:, :], m8b[:, :], zap[:, :], NEG)

                # compact: packed = 2^20 * is_marked + gj, top-16 of packed
                mask = mpool.tile([128, nco], f32, tag="mask")
                nc.vector.tensor_scalar(
                    mask[:, :], zap[:, :], -1.0e38, MARK,
                    op0=mybir.AluOpType.is_le, op1=mybir.AluOpType.mult,
                )
                nc.vector.tensor_add(mask[:, :], mask[:, :], gj[:, :])
                p8a = mpool.tile([128, 8], f32, tag="p8a")
                p8b = mpool.tile([128, 8], f32, tag="p8b")
                nc.vector.max(p8a[:, :], mask[:, :])
                nc.vector.match_replace(mask[:, :], p8a[:, :], mask[:, :], NEG)
                nc.vector.max(p8b[:, :], mask[:, :])

                jall_b = mpool.tile([128, k], f32, tag="jall")
                nc.scalar.activation(
                    jall_b[:, 0:8], p8a[:, :],
                    mybir.ActivationFunctionType.Copy, bias=-MARK,
                )
                nc.scalar.activation(
                    jall_b[:, 8:16], p8b[:, :],
                    mybir.ActivationFunctionType.Copy, bias=-MARK,
                )
                jall_tiles[b] = jall_b

            # PE-transpose block b's [128 rows, 16] f32 index tile to
            # [16, 128], convert to i16 on ACT, replicate into all 8 Q7 idx
            # groups (HW reads every group), then issue the 4 gathers.
            def jrep_and_gather(b):
                jt = ppJ.tile([k, BLK], f32, tag="jt")
                nc.tensor.transpose(jt[:, :], jall_tiles.pop(b)[:, :], ident[:, :])
                jts = mpool.tile([k, BLK], i16, tag="jts")
                nc.scalar.activation(
                    jts[:, :], jt[:, :], mybir.ActivationFunctionType.Copy
                )
                # last block's replication is on the critical tail: spread it
                # across idle queues; otherwise keep everything on SP so the
                # ACT/Pool queues stay clear for merge/gather work
                qeng = (
                    [nc.sync, nc.scalar, nc.gpsimd]
                    if b == nblk - 1 else [nc.sync]
                )
                for q in range(8):
                    qeng[q % len(qeng)].dma_start(
                        jrep[b][16 * q:16 * q + k, :], jts[:, :]
                    )
                xj = dpool.tile([128, BLK * k], f16, tag="xj")
                for s in range(subs):
                    nc.gpsimd.dma_gather(
                        xj[:, s * MMW:(s + 1) * MMW].rearrange(
                            "p (o e) -> p o e", o=1
                        ),
                        xpad_d[:, :],
                        jrep[b][:, s * rows_per_chunk:(s + 1) * rows_per_chunk],
                        MMW,
                        MMW,
                        c,
                        transpose=True,
                    )
                return xj

            def mlp_mms(b, xj):
                h2 = dpool.tile([128, BLK * k], f16, tag="h2")
                for s in range(subs):
                    e0 = s * MMW
                    ps = ppM.tile([128, MMW], f32, tag="mm")
                    nc.tensor.matmul(
                        ps[:, :], w1b16[:, :], xj[:, e0:e0 + MMW],
                        start=True, stop=False,
                    )
                    nc.tensor.matmul(
                        ps[:, :],
                        prow16[:, (b * subs + s) * c:(b * subs + s + 1) * c],
                        ssel[:, :],
                        start=False, stop=True,
                    )
                    h1 = dpool.tile([128, MMW], f16, tag="h1")
                    nc.scalar.activation(
                        h1[:, :], ps[:, :], mybir.ActivationFunctionType.Relu,
                        bias=b1[:, :],
                    )
                    ps2 = ppM.tile([128, MMW], f32, tag="mm")
                    nc.tensor.matmul(ps2[:, :], w2[:, :], h1[:, :])
                    nc.scalar.activation(
                        h2[:, e0:e0 + MMW], ps2[:, :],
                        mybir.ActivationFunctionType.Relu,
                        bias=b2s[:, :], scale=1.0 / k,
                    )
                return h2

            def mlp_tree(b, h2):
                # mean over k=16 via fp16 TT-add tree (2x DVE mode)
                h2v = h2[:, :].rearrange("p (r k) -> p r k", k=k)
                t1 = dpool.tile([128, BLK * 8], f16, tag="t1")
                t1v = t1[:, :].rearrange("p (r k) -> p r k", k=8)
                nc.vector.tensor_tensor(
                    out=t1v, in0=h2v[:, :, 0:8], in1=h2v[:, :, 8:16],
                    op=mybir.AluOpType.add,
                )
                t2 = dpool.tile([128, BLK * 4], f16, tag="t2")
                t2v = t2[:, :].rearrange("p (r k) -> p r k", k=4)
                nc.vector.tensor_tensor(
                    out=t2v, in0=t1v[:, :, 0:4], in1=t1v[:, :, 4:8],
                    op=mybir.AluOpType.add,
                )
                t3 = dpool.tile([128, BLK * 2], f16, tag="t3")
                t3v = t3[:, :].rearrange("p (r k) -> p r k", k=2)
                nc.vector.tensor_tensor(
                    out=t3v, in0=t2v[:, :, 0:2], in1=t2v[:, :, 2:4],
                    op=mybir.AluOpType.add,
                )
                o_v = outT[:, b * BLK:(b + 1) * BLK].rearrange(
                    "p (r k) -> p r k", k=1
                )
                nc.vector.tensor_tensor(
                    out=o_v, in0=t3v[:, :, 0:1], in1=t3v[:, :, 1:2],
                    op=mybir.AluOpType.add,
                )

            def out_dma(b):
                nc.sync.dma_start(
                    out_d[:, b * BLK:(b + 1) * BLK], outT[:, b * BLK:(b + 1) * BLK]
                )

            # Pipeline (offset 2): iter b runs the MLP matmuls of block b-2
            # (whose gather landed during iter b-1) first — they never wait —
            # then block b's screen; block b-1's transpose + gathers go at
            # the iteration tail (transpose input came from merge(b-1) last
            # iteration, so the PE FIFO again never stalls on them).
            xj_pending = {}
            for b in range(nblk):
                if b == 0:
                    # warm the PE ramp while xaug's first slice loads
                    phase_p(b)
                h2_prev = None
                if b >= 2:
                    h2_prev = mlp_mms(b - 2, xj_pending.pop(b - 2))
                screen_windows(b, 0, 2)
                if b > 0:
                    phase_p(b)
                if h2_prev is not None:
                    mlp_tree(b - 2, h2_prev)
                    out_dma(b - 2)
                screen_windows(b, 2, nwin)
                merge(b)
                if b >= 1:
                    xj_pending[b - 1] = jrep_and_gather(b - 1)
            b2 = nblk - 2
            mlp_tree(b2, mlp_mms(b2, xj_pending.pop(b2)))
            out_dma(b2)
            blast = nblk - 1
            mlp_tree(blast, mlp_mms(blast, jrep_and_gather(blast)))
            out_dma(blast)

    nc.finalize()
    return nc


def host_prep(x, W1, b1, W2, b2, n=N, d=D, c=C, k=K, rpc=RPC, ncores=NCORES):
    x = np.ascontiguousarray(np.asarray(x, dtype=np.float32))
    W1 = np.asarray(W1, dtype=np.float32)
    b1 = np.asarray(b1, dtype=np.float32)
    W2 = np.asarray(W2, dtype=np.float32)
    b2 = np.asarray(b2, dtype=np.float32)

    sq = np.sum(x * x, axis=1, dtype=np.float32)
    da = d + 2
    nwin = n // WIN
    nco = nwin * 8
    rows_per_chunk = MMW // k

    xaug = np.zeros((da, n), dtype=np.float32)
    xaug[:d] = x.T
    xaug[d] = sq

    w1dh = ((W1[:d] - W1[d:]) * 0.5).astype(np.float32)
    w1b16 = np.zeros((c, c), dtype=np.float16)
    w1b16[:d] = W1[d:].astype(np.float16)
    w2 = W2.astype(np.float16)
    ssel = np.zeros((rows_per_chunk, MMW), dtype=np.float16)
    for r in range(rows_per_chunk):
        ssel[r, r * k:(r + 1) * k] = 1.0
    b1c = b1.reshape(c, 1).astype(np.float32)
    b2s = (b2 / k).reshape(c, 1).astype(np.float32)
    identf = np.eye(128, dtype=np.float32)
    xpad16 = np.zeros((n, c), dtype=np.float16)
    xpad16[:, :d] = x.astype(np.float16)

    in_maps = []
    for cid in range(ncores):
        rows = x[cid * rpc:(cid + 1) * rpc]
        wloc = np.empty((da, rpc), dtype=np.float32)
        wloc[:d] = 2.0 * rows.T
        wloc[d:] = -1.0
        in_maps.append(
            dict(
                xaug=xaug, wloc=np.ascontiguousarray(wloc), w1dh=w1dh,
                w1b16=w1b16, w2=w2, ssel=ssel, b1c=b1c, b2s=b2s,
                identf=identf, xpad16=xpad16,
            )
        )
    return in_maps


_NC_CACHE = {}


def kernel(x, W1, b1, W2, b2):
    _ensure_path()
    from concourse.bass_utils import run_bass_kernel_spmd

    key = "full"
    if key not in _NC_CACHE:
        _NC_CACHE[key] = build_program()
    nc = _NC_CACHE[key]

    in_maps = host_prep(x, W1, b1, W2, b2)
    res = run_bass_kernel_spmd(
        nc, in_maps, core_ids=list(range(NCORES)),
        trace=bool(int(os.environ.get("DGCNN_TRACE", "0"))),
    )
    out = np.empty((N, C), dtype=np.float32)
    for cid in range(NCORES):
        out[cid * RPC:(cid + 1) * RPC] = res.results[cid]["outT"].T
    if getattr(res, "exec_time_ns", None):
        kernel.last_exec_time_ns = res.exec_time_ns
    return out


kernel.last_exec_time_ns = None
